# revision 20
# baseline (speedup 1.0000x reference)
"""Trainium2 Bass kernel for CapsuleLayer dynamic routing (B=128, I=1152, J=128, K=32, D=32).

Strategy
--------
Data-parallel over batch: 16 samples per core x 8 cores. The routing math is
algebraically factorized so u_hat [B,I,K,D] (604 MB) is never materialized:

    y[s,k,j]  = sum_i c[s,i,k] x[s,i,j]          (per-sample PE matmul, i contracted)
    s[s,k,d]  = sum_j y[s,k,j] W[j,k,d]          (per-k PE matmul, j contracted)
    v         = squash(s)                         (DVE/ACT elementwise)
    t[s,j,k]  = sum_d W[j,k,d] v[s,k,d]          (per-k PE matmul, d contracted)
    b[s,i,k] += sum_j x[s,i,j] t[s,j,k]          (per-sample PE matmul, j contracted)

x is staged in two on-chip layouts (i-major xa for y, j-major xb for the b
update), both prepared host-side in bf16. Perf-critical structure (from trace
analysis):
  - all big matmuls keep CONTIGUOUS moving operands (strided moving costs 60ns
    vs 27ns per instr): cs is k-inner; the t-matmul writes its PSUM output
    pre-transposed ([j, s, k]) so T2[:, s, :] is flat for the b-update.
  - t-matmuls run in bf16 (f32r stationary ldweights cost 328ns each).
  - iteration 1's agreement update never touches b again: iteration 2 uses
    exp(b0 + db) = exp(b0) * exp(db), with exp(db) read straight out of the
    b-update PSUM tile by the ACT engine per sample — no PSUM->SBUF copies or
    adds on the DVE.
  - squash's two k-halves are op-interleaved so ACT/DVE pipeline, and the
    Sqrt table load hides under the DVE reduce.
  - softmax normalize mult runs on DVE for the schedule-critical groups
    (g0, g3) and GpSimd for the middle ones.
  - x DMAs ride 3 hardware queues (sync/gpsimd/scalar); all xb tiles land
    first (iteration 0's b-update + iteration 1's softmax complete early),
    then xa group by group (iteration 1's y rides the tail).
Iteration 0's softmax of zeros is uniform, so its y reduces to sum_i x / K,
computed host-side in fp32 (y0).
"""
import numpy as np
import ml_dtypes
from contextlib import ExitStack

import concourse.bass as bass
import concourse.bacc as bacc_mod
import concourse.mybir as mybir
import concourse.tile as tile
from concourse.bass_utils import run_bass_kernel_spmd
from concourse.masks import make_identity

B, I, J, K, D = 128, 1152, 128, 32, 32
NCORES = 8
S = B // NCORES          # 16 samples per core
CH = I // 128            # 9 chunks of the input-capsule axis
NUM_ROUTING = 3
EPS = 1e-7
F32 = mybir.dt.float32
F32R = mybir.dt.float32r
BF16 = mybir.dt.bfloat16

_PROGRAM = None

SG = 4  # softmax sample-group size
NG = S // SG


def _squash2(nc, sqpool, s_ps, vsb, eps_t, quarter_done=None):
    """vsb = squash(s_ps) along d. The PSUM Square runs in two k-halves on
    ACT; everything downstream runs in four k-quarters so the ACT/DVE chains
    pipeline (and the Sqrt table load hides under the reduces).
    quarter_done(k0, k1) is called after each quarter's vsb slice is final."""
    sq = {}
    for k0 in (0, K // 2):
        k1 = k0 + K // 2
        sq[k0] = sqpool.tile([S, K // 2, D], F32, tag=f"sq{k0}", name=f"sq{k0}")
        nc.scalar.activation(out=sq[k0], in_=s_ps[:, k0:k1, :],
                             func=mybir.ActivationFunctionType.Square)
    KQ = K // 4
    quarters = [(q * KQ, (q + 1) * KQ) for q in range(4)]
    ss, rt = {}, {}
    for k0, k1 in quarters:
        sqh = sq[0 if k0 < K // 2 else K // 2]
        o = k0 % (K // 2)
        ss[k0] = sqpool.tile([S, KQ], F32, tag=f"ss{k0}", name=f"ss{k0}")
        nc.vector.tensor_reduce(out=ss[k0], in_=sqh[:, o:o + KQ],
                                axis=mybir.AxisListType.X, op=mybir.AluOpType.add)
    for k0, k1 in quarters:
        rt[k0] = sqpool.tile([S, KQ], F32, tag=f"rt{k0}", name=f"rt{k0}")
        nc.scalar.activation(out=rt[k0], in_=ss[k0],
                             func=mybir.ActivationFunctionType.Sqrt, bias=eps_t)
    for k0, k1 in quarters:
        den = sqpool.tile([S, KQ], F32, tag=f"den{k0}", name=f"den{k0}")
        nc.vector.scalar_tensor_tensor(out=den, in0=ss[k0], scalar=1.0,
                                       in1=rt[k0], op0=mybir.AluOpType.add,
                                       op1=mybir.AluOpType.mult)
        rden = sqpool.tile([S, KQ], F32, tag=f"rden{k0}", name=f"rden{k0}")
        nc.vector.reciprocal(out=rden, in_=den)
        sc = sqpool.tile([S, KQ], F32, tag=f"sc{k0}", name=f"sc{k0}")
        nc.vector.tensor_mul(sc, ss[k0], rden)
        nc.vector.tensor_tensor(out=vsb[:, k0:k1, :], in0=s_ps[:, k0:k1, :],
                                in1=sc.unsqueeze(-1).broadcast_to([S, KQ, D]),
                                op=mybir.AluOpType.mult)
        if quarter_done is not None:
            quarter_done(vsb, k0, k1)


def _build_program():
    nc = bacc_mod.Bacc("TRN2", target_bir_lowering=False, debug=False,
                       num_devices=NCORES)
    xa_d = nc.dram_tensor("xa", [128, S, CH, 128], BF16, kind="ExternalInput")
    xb_d = nc.dram_tensor("xb", [128, S, CH * 128], BF16, kind="ExternalInput")
    wr_d = nc.dram_tensor("wr", [128, K, D], F32R, kind="ExternalInput")
    wt_d = nc.dram_tensor("wt", [32, K, 128], BF16, kind="ExternalInput")
    y0_d = nc.dram_tensor("y0", [128, S], F32R, kind="ExternalInput")
    v_d = nc.dram_tensor("vout", [S, K, D], F32, kind="ExternalOutput")

    with tile.TileContext(nc) as tc, ExitStack() as ctx:
        const = ctx.enter_context(tc.tile_pool(name="const", bufs=1))
        xap = ctx.enter_context(tc.tile_pool(name="xa", bufs=1))
        xbp = ctx.enter_context(tc.tile_pool(name="xb", bufs=1))
        bp = ctx.enter_context(tc.tile_pool(name="b", bufs=1))
        epool = ctx.enter_context(tc.tile_pool(name="e1", bufs=1))
        edp = ctx.enter_context(tc.tile_pool(name="ed", bufs=1))
        cpool = ctx.enter_context(tc.tile_pool(name="c", bufs=1))
        zpool = ctx.enter_context(tc.tile_pool(name="z", bufs=1))
        y2p = ctx.enter_context(tc.tile_pool(name="y2", bufs=2))
        t2p = ctx.enter_context(tc.tile_pool(name="t2", bufs=2))
        vt2p = ctx.enter_context(tc.tile_pool(name="vt2", bufs=2))
        vp = ctx.enter_context(tc.tile_pool(name="v", bufs=1))
        sqp = ctx.enter_context(tc.tile_pool(name="sqp", bufs=1))
        ps_ys = ctx.enter_context(tc.tile_pool(name="ps_ys", bufs=1, space="PSUM"))
        ps_vt = ctx.enter_context(tc.tile_pool(name="ps_vt", bufs=1, space="PSUM"))
        ps_t = ctx.enter_context(tc.tile_pool(name="ps_t", bufs=1, space="PSUM"))
        ps_b = ctx.enter_context(tc.tile_pool(name="ps_b", bufs=3, space="PSUM"))

        # constants: small, needed first for iteration 0's s/t chain.
        y0t = const.tile([128, S], F32R)
        nc.sync.dma_start(out=y0t, in_=y0_d[:])
        wr = const.tile([128, K, D], F32R)
        nc.gpsimd.dma_start(out=wr, in_=wr_d[:])
        wt = const.tile([32, K, 128], BF16)
        nc.scalar.dma_start(out=wt, in_=wt_d[:])
        ident = const.tile([S, S], F32)
        make_identity(nc, ident)
        eps_t = const.tile([S, 1], F32)
        nc.vector.memset(eps_t, EPS)

        # x loads: ALL xb first (b-update it0 finishes early, freeing DVE/ACT
        # before iteration 1's softmax), then xa group by group. Only the
        # sync/gpsimd queues carry x tiles — a full DGE ring stalls the
        # issuing engine, and the scalar (ACT) engine has early squash work.
        dq = [nc.sync, nc.gpsimd]
        xb_s = [None] * S
        xa_s = [None] * S
        for s in range(S):
            tb = xbp.tile([128, CH * 128], BF16, tag=f"xb{s}")
            dq[s % 2].dma_start(out=tb, in_=xb_d[:, s])
            xb_s[s] = tb
        for s in range(S):
            ta = xap.tile([128, CH, 128], BF16, tag=f"xa{s}")
            dq[s % 2].dma_start(out=ta, in_=xa_d[:, s])
            xa_s[s] = ta

        btile = bp.tile([128, S, CH, K], F32)
        wrf = wr.rearrange("p k d -> p (k d)")

        def softmax_group(g, mult_eng, e_save=None):
            """softmax over k of b[:, gSG:(g+1)SG] -> bf16 cs tile.
            If e_save is given, exp() lands there (kept for iteration 2) and
            the normalized cs goes to a separate tile."""
            bsl = btile[:, g * SG:(g + 1) * SG]
            if e_save is not None:
                e = e_save
            else:
                e = cpool.tile([128, SG, CH, K], BF16, tag=f"cs{g}")
            nc.scalar.activation(out=e, in_=bsl,
                                 func=mybir.ActivationFunctionType.Exp)
            return normalize(e, g, mult_eng, out_sep=e_save is not None)

        def normalize(e, g, mult_eng, out_sep, red_gps=False):
            z = zpool.tile([128, SG, CH], F32, tag=f"z{g}")
            if red_gps:
                # halving-tree add on GpSimd (bf16 intermediates) frees the
                # DVE of the 1.35us reduce in the fused region
                cur, w = e, K
                while w > 2:
                    w //= 2
                    nxt = zpool.tile([128, SG, CH, w], BF16, tag=f"zt{g}_{w}",
                                     name=f"zt{w}")
                    nc.gpsimd.tensor_tensor(out=nxt, in0=cur[:, :, :, 0:w],
                                            in1=cur[:, :, :, w:2 * w],
                                            op=mybir.AluOpType.add)
                    cur = nxt
                nc.gpsimd.tensor_tensor(out=z.unsqueeze(-1), in0=cur[:, :, :, 0:1],
                                        in1=cur[:, :, :, 1:2],
                                        op=mybir.AluOpType.add)
            else:
                nc.vector.tensor_reduce(out=z, in_=e, axis=mybir.AxisListType.X,
                                        op=mybir.AluOpType.add)
            r = zpool.tile([128, SG, CH], F32, tag=f"r{g}")
            nc.vector.reciprocal(out=r, in_=z)
            if out_sep:
                cs = cpool.tile([128, SG, CH, K], BF16, tag=f"cs{g}")
            else:
                cs = e
            mult_eng.tensor_tensor(out=cs, in0=e,
                                   in1=r.unsqueeze(-1).broadcast_to([128, SG, CH, K]),
                                   op=mybir.AluOpType.mult)
            return cs

        def s_matmuls(Y2):
            s_ps = ps_ys.tile([S, K, D], F32, tag="s")
            for k in range(K):
                nc.tensor.matmul(s_ps[:, k, :], Y2[:, :, k], wr[:, k, :],
                                 start=True, stop=True)
            return s_ps

        def squash_to_v(s_ps, quarter_done=None):
            vsb = vp.tile([S, K, D], F32, tag="v")
            _squash2(nc, sqp, s_ps, vsb, eps_t, quarter_done)
            return vsb

        def v_to_T2(vsb):
            # vT[d, k, s] via PE transposes; t-matmuls write [j, s, k] PSUM
            # directly (strided out) so the T2 cast is contiguous and the
            # b-update moving operand T2[:, s, :] is flat.
            vt_ps = ps_vt.tile([32, K, S], F32, tag="vt")
            for k in range(K):
                nc.tensor.transpose(out=vt_ps[:, k, :], in_=vsb[:, k, :],
                                    identity=ident)
            vt2 = vt2p.tile([32, K, S], BF16, tag="vt2")
            nc.scalar.copy(out=vt2, in_=vt_ps)
            t_ps = ps_t.tile([128, S, K], F32, tag="t")
            for k in range(K):
                nc.tensor.matmul(t_ps[:, :, k], wt[:, k, :],
                                 vt2[:, k, :], start=True, stop=True)
            T2 = t2p.tile([128, S, K], BF16, tag="T2")
            nc.vector.tensor_copy(out=T2, in_=t_ps)
            return T2

        def b_matmuls(s, T2):
            bu = ps_b.tile([128, CH, K], F32, tag="bu")
            for ic in range(CH):
                nc.tensor.matmul(bu[:, ic, :],
                                 xb_s[s][:, ic * 128:(ic + 1) * 128],
                                 T2[:, s, :], start=True, stop=True)
            return bu

        def y_matmuls(y_ps, cs, g):
            for si in range(SG):
                s = g * SG + si
                for ic in range(CH):
                    nc.tensor.matmul(y_ps[:, s, :], xa_s[s][:, ic, :],
                                     cs[:, si, ic, :],
                                     start=(ic == 0), stop=(ic == CH - 1))

        # ---------------- iteration 0 ----------------
        s_ps = ps_ys.tile([S, K, D], F32, tag="s")
        s_flat = s_ps.rearrange("s k d -> s (k d)")
        nc.tensor.matmul(s_flat[:, :512], y0t, wrf[:, :512], start=True, stop=True)
        nc.tensor.matmul(s_flat[:, 512:], y0t, wrf[:, 512:], start=True, stop=True)
        vsb = squash_to_v(s_ps)
        T2 = v_to_T2(vsb)
        for s in range(S):
            bu = b_matmuls(s, T2)
            dst = btile[:, s].rearrange("p c k -> p (c k)")
            buf = bu.rearrange("p c k -> p (c k)")
            # copies alternate DVE/ACT so the PE never waits on one engine
            if s % 2 == 0:
                nc.vector.tensor_copy(out=dst, in_=buf)
            else:
                nc.scalar.copy(out=dst, in_=buf)

        # ---------------- iteration 1: softmax + y, then s/squash/t ----------
        # normalize mult: GpSimd for the early groups (latency hides under
        # earlier work), DVE for the schedule-critical late groups.
        mult_engs = [nc.gpsimd, nc.gpsimd, nc.vector, nc.vector]
        y_ps = ps_ys.tile([128, S, K], F32, tag="y")
        e1 = []
        for g in range(NG):
            e = epool.tile([128, SG, CH, K], BF16, tag=f"e1{g}")
            cs = softmax_group(g, mult_engs[g], e_save=e)
            e1.append(e)
            y_matmuls(y_ps, cs, g)
        Y2 = y2p.tile([128, S, K], F32R, tag="Y2")
        nc.vector.tensor_copy(out=Y2, in_=y_ps)
        s_ps = s_matmuls(Y2)
        vsb = squash_to_v(s_ps)
        T2 = v_to_T2(vsb)

        # ------- iteration 1 b-update fused with iteration 2 softmax --------
        # exp(b0 + db) = exp(b0) * exp(db): ACT exps each sample's b-update
        # delta straight out of PSUM; one pure-bf16 2x DVE mult per group
        # recovers the unnormalized e for iteration 2. No adds, no copies.
        y_ps2 = ps_ys.tile([128, S, K], F32, tag="y")
        cs_q = []
        for g in range(NG):
            ed = edp.tile([128, SG, CH, K], BF16, tag=f"ed{g}")
            for si in range(SG):
                bu = b_matmuls(g * SG + si, T2)
                nc.scalar.activation(out=ed[:, si], in_=bu,
                                     func=mybir.ActivationFunctionType.Exp)
            e2 = cpool.tile([128, SG, CH, K], BF16, tag=f"cs{g}")
            nc.vector.tensor_tensor(out=e2, in0=e1[g], in1=ed,
                                    op=mybir.AluOpType.mult)
            cs_q.append(normalize(e2, g, mult_engs[g], out_sep=False,
                                  red_gps=(g < 2)))
            if g >= 1:
                y_matmuls(y_ps2, cs_q[g - 1], g - 1)
        y_matmuls(y_ps2, cs_q[NG - 1], NG - 1)

        # ---------------- iteration 2 tail: s, squash, output ----------------
        Y2b = y2p.tile([128, S, K], F32R, tag="Y2")
        nc.vector.tensor_copy(out=Y2b, in_=y_ps2)
        s_ps = s_matmuls(Y2b)
        outq = [nc.sync, nc.gpsimd]

        def stream_out(vt, k0, k1):
            outq[(k0 // (K // 4)) % 2].dma_start(out=v_d[:, k0:k1],
                                                 in_=vt[:, k0:k1])
        vsb = squash_to_v(s_ps, quarter_done=stream_out)

    nc.compile()
    return nc


def _get_program():
    global _PROGRAM
    if _PROGRAM is None:
        _PROGRAM = _build_program()
    return _PROGRAM


def _prep_core_inputs(x_core, wr, wt):
    """x_core: [S, I, J] fp32 -> per-core input map."""
    bf = ml_dtypes.bfloat16
    xa = np.ascontiguousarray(
        x_core.reshape(S, CH, 128, J).transpose(2, 0, 1, 3).astype(bf))  # [128,S,CH,J]
    xb = np.ascontiguousarray(x_core.transpose(2, 0, 1).astype(bf))      # [J,S,I]
    y0 = np.ascontiguousarray((x_core.sum(axis=1) / K).T)                # [J,S] f32
    return {"xa": xa, "xb": xb.reshape(J, S, CH * 128), "wr": wr,
            "wt": np.ascontiguousarray(wt.astype(bf)), "y0": y0}


def kernel(inputs, W):
    x = np.ascontiguousarray(np.asarray(inputs, dtype=np.float32))
    Wf = np.ascontiguousarray(np.asarray(W, dtype=np.float32))           # [J, K, D]
    wt = np.ascontiguousarray(Wf.transpose(2, 1, 0))                     # [D, K, J]
    nc = _get_program()
    in_maps = [_prep_core_inputs(x[c * S:(c + 1) * S], Wf, wt) for c in range(NCORES)]
    res = run_bass_kernel_spmd(nc, in_maps, list(range(NCORES)))
    return np.concatenate([r["vout"] for r in res.results], axis=0)


# revision 21
# speedup vs baseline: 1.0328x; 1.0328x over previous
"""Trainium2 Bass kernel for CapsuleLayer dynamic routing (B=128, I=1152, J=128, K=32, D=32).

Strategy
--------
Data-parallel over batch: 16 samples per core x 8 cores. The routing math is
algebraically factorized so u_hat [B,I,K,D] (604 MB) is never materialized:

    y[s,k,j]  = sum_i c[s,i,k] x[s,i,j]          (per-sample PE matmul, i contracted)
    s[s,k,d]  = sum_j y[s,k,j] W[j,k,d]          (per-k PE matmul, j contracted)
    v         = squash(s)                         (DVE/ACT elementwise)
    t[s,j,k]  = sum_d W[j,k,d] v[s,k,d]          (per-k PE matmul, d contracted)
    b[s,i,k] += sum_j x[s,i,j] t[s,j,k]          (per-sample PE matmul, j contracted)

x is staged in two on-chip layouts (i-major xa for y, j-major xb for the b
update), both prepared host-side in bf16. Perf-critical structure (from trace
analysis):
  - all big matmuls keep CONTIGUOUS moving operands (strided moving costs 60ns
    vs 27ns per instr): cs is k-inner; the t-matmul writes its PSUM output
    pre-transposed ([j, s, k]) so T2[:, s, :] is flat for the b-update.
  - t-matmuls run in bf16 (f32r stationary ldweights cost 328ns each).
  - iteration 1's agreement update never touches b again: iteration 2 uses
    exp(b0 + db) = exp(b0) * exp(db), with exp(db) read straight out of the
    b-update PSUM tile by the ACT engine per sample — no PSUM->SBUF copies or
    adds on the DVE.
  - squash's two k-halves are op-interleaved so ACT/DVE pipeline, and the
    Sqrt table load hides under the DVE reduce.
  - softmax normalize mult runs on DVE for the schedule-critical groups
    (g0, g3) and GpSimd for the middle ones.
  - x DMAs ride 3 hardware queues (sync/gpsimd/scalar); all xb tiles land
    first (iteration 0's b-update + iteration 1's softmax complete early),
    then xa group by group (iteration 1's y rides the tail).
Iteration 0's softmax of zeros is uniform, so its y reduces to sum_i x / K,
computed host-side in fp32 (y0).
"""
import numpy as np
import ml_dtypes
from contextlib import ExitStack

import concourse.bass as bass
import concourse.bacc as bacc_mod
import concourse.mybir as mybir
import concourse.tile as tile
from concourse.bass_utils import run_bass_kernel_spmd
from concourse.masks import make_identity

B, I, J, K, D = 128, 1152, 128, 32, 32
NCORES = 8
S = B // NCORES          # 16 samples per core
CH = I // 128            # 9 chunks of the input-capsule axis
NUM_ROUTING = 3
EPS = 1e-7
F32 = mybir.dt.float32
F32R = mybir.dt.float32r
BF16 = mybir.dt.bfloat16

_PROGRAM = None

SG = 4  # softmax sample-group size
NG = S // SG


def _squash2(nc, sqpool, s_ps, vsb, eps_t, quarter_done=None):
    """vsb = squash(s_ps) along d. The PSUM Square runs in two k-halves on
    ACT; everything downstream runs in four k-quarters so the ACT/DVE chains
    pipeline (and the Sqrt table load hides under the reduces).
    quarter_done(k0, k1) is called after each quarter's vsb slice is final."""
    sq = {}
    for k0 in (0, K // 2):
        k1 = k0 + K // 2
        sq[k0] = sqpool.tile([S, K // 2, D], F32, tag=f"sq{k0}", name=f"sq{k0}")
        nc.scalar.activation(out=sq[k0], in_=s_ps[:, k0:k1, :],
                             func=mybir.ActivationFunctionType.Square)
    KQ = K // 4
    quarters = [(q * KQ, (q + 1) * KQ) for q in range(4)]
    ss, rt = {}, {}
    for k0, k1 in quarters:
        sqh = sq[0 if k0 < K // 2 else K // 2]
        o = k0 % (K // 2)
        ss[k0] = sqpool.tile([S, KQ], F32, tag=f"ss{k0}", name=f"ss{k0}")
        nc.vector.tensor_reduce(out=ss[k0], in_=sqh[:, o:o + KQ],
                                axis=mybir.AxisListType.X, op=mybir.AluOpType.add)
    for k0, k1 in quarters:
        rt[k0] = sqpool.tile([S, KQ], F32, tag=f"rt{k0}", name=f"rt{k0}")
        nc.scalar.activation(out=rt[k0], in_=ss[k0],
                             func=mybir.ActivationFunctionType.Sqrt, bias=eps_t)
    for k0, k1 in quarters:
        den = sqpool.tile([S, KQ], F32, tag=f"den{k0}", name=f"den{k0}")
        nc.vector.scalar_tensor_tensor(out=den, in0=ss[k0], scalar=1.0,
                                       in1=rt[k0], op0=mybir.AluOpType.add,
                                       op1=mybir.AluOpType.mult)
        rden = sqpool.tile([S, KQ], F32, tag=f"rden{k0}", name=f"rden{k0}")
        nc.vector.reciprocal(out=rden, in_=den)
        sc = sqpool.tile([S, KQ], F32, tag=f"sc{k0}", name=f"sc{k0}")
        nc.vector.tensor_mul(sc, ss[k0], rden)
        nc.vector.tensor_tensor(out=vsb[:, k0:k1, :], in0=s_ps[:, k0:k1, :],
                                in1=sc.unsqueeze(-1).broadcast_to([S, KQ, D]),
                                op=mybir.AluOpType.mult)
        if quarter_done is not None:
            quarter_done(vsb, k0, k1)


def _build_program():
    nc = bacc_mod.Bacc("TRN2", target_bir_lowering=False, debug=False,
                       num_devices=NCORES)
    xa_d = nc.dram_tensor("xa", [128, S, CH, 128], BF16, kind="ExternalInput")
    xb_d = nc.dram_tensor("xb", [128, S, CH * 128], BF16, kind="ExternalInput")
    wr_d = nc.dram_tensor("wr", [128, K, D], F32R, kind="ExternalInput")
    wt_d = nc.dram_tensor("wt", [32, K, 128], BF16, kind="ExternalInput")
    y0_d = nc.dram_tensor("y0", [128, S], F32R, kind="ExternalInput")
    v_d = nc.dram_tensor("vout", [S, K, D], F32, kind="ExternalOutput")

    with tile.TileContext(nc) as tc, ExitStack() as ctx:
        const = ctx.enter_context(tc.tile_pool(name="const", bufs=1))
        cpoolA = ctx.enter_context(tc.tile_pool(name="cA", bufs=1))
        xap = ctx.enter_context(tc.tile_pool(name="xa", bufs=1))
        xbp = ctx.enter_context(tc.tile_pool(name="xb", bufs=1))
        bp = ctx.enter_context(tc.tile_pool(name="b", bufs=1))
        epool = ctx.enter_context(tc.tile_pool(name="e1", bufs=1))
        edp = ctx.enter_context(tc.tile_pool(name="ed", bufs=1))
        zpool = ctx.enter_context(tc.tile_pool(name="z", bufs=1))
        y2p = ctx.enter_context(tc.tile_pool(name="y2", bufs=2))
        t2p = ctx.enter_context(tc.tile_pool(name="t2", bufs=2))
        vt2p = ctx.enter_context(tc.tile_pool(name="vt2", bufs=2))
        vp = ctx.enter_context(tc.tile_pool(name="v", bufs=1))
        sqp = ctx.enter_context(tc.tile_pool(name="sqp", bufs=1))
        cpoolB = ctx.enter_context(tc.tile_pool(name="cB", bufs=1))
        ps_ys = ctx.enter_context(tc.tile_pool(name="ps_ys", bufs=1, space="PSUM"))
        ps_vt = ctx.enter_context(tc.tile_pool(name="ps_vt", bufs=1, space="PSUM"))
        ps_t = ctx.enter_context(tc.tile_pool(name="ps_t", bufs=1, space="PSUM"))
        ps_b = ctx.enter_context(tc.tile_pool(name="ps_b", bufs=3, space="PSUM"))

        # constants: small, needed first for iteration 0's s/t chain.
        y0t = const.tile([128, S], F32R)
        nc.sync.dma_start(out=y0t, in_=y0_d[:])
        wr = const.tile([128, K, D], F32R)
        nc.gpsimd.dma_start(out=wr, in_=wr_d[:])
        wt = const.tile([32, K, 128], BF16)
        nc.scalar.dma_start(out=wt, in_=wt_d[:])
        ident = const.tile([S, S], F32)
        make_identity(nc, ident)
        eps_t = const.tile([S, 1], F32)
        nc.vector.memset(eps_t, EPS)

        # x loads: ALL xb first (b-update it0 finishes early, freeing DVE/ACT
        # before iteration 1's softmax), then xa group by group. Only the
        # sync/gpsimd queues carry x tiles — a full DGE ring stalls the
        # issuing engine, and the scalar (ACT) engine has early squash work.
        dq = [nc.sync, nc.gpsimd]
        xb_s = [None] * S
        xa_s = [None] * S
        for s in range(S):
            tb = xbp.tile([128, CH * 128], BF16, tag=f"xb{s}")
            dq[s % 2].dma_start(out=tb, in_=xb_d[:, s])
            xb_s[s] = tb
        for s in range(S):
            ta = xap.tile([128, CH, 128], BF16, tag=f"xa{s}")
            dq[s % 2].dma_start(out=ta, in_=xa_d[:, s])
            xa_s[s] = ta

        btile = bp.tile([128, S, CH, K], F32)
        wrf = wr.rearrange("p k d -> p (k d)")

        def softmax_group(g, mult_eng, e_save=None):
            """softmax over k of b[:, gSG:(g+1)SG] -> bf16 cs tile.
            If e_save is given, exp() lands there (kept for iteration 2) and
            the normalized cs goes to a separate tile."""
            bsl = btile[:, g * SG:(g + 1) * SG]
            if e_save is not None:
                e = e_save
            else:
                e = (cpoolA if g % 2 == 0 else cpoolB).tile(
                    [128, SG, CH, K], BF16, tag=f"cs{g}", name=f"cs{g}")
            nc.scalar.activation(out=e, in_=bsl,
                                 func=mybir.ActivationFunctionType.Exp)
            return normalize(e, g, mult_eng, out_sep=e_save is not None)

        def normalize(e, g, mult_eng, out_sep, red_gps=False):
            z = zpool.tile([128, SG, CH], F32, tag=f"z{g}")
            if red_gps:
                # halving-tree add on GpSimd (bf16 intermediates) frees the
                # DVE of the 1.35us reduce in the fused region
                cur, w = e, K
                while w > 2:
                    w //= 2
                    nxt = zpool.tile([128, SG, CH, w], BF16, tag=f"zt{g}_{w}",
                                     name=f"zt{w}")
                    nc.gpsimd.tensor_tensor(out=nxt, in0=cur[:, :, :, 0:w],
                                            in1=cur[:, :, :, w:2 * w],
                                            op=mybir.AluOpType.add)
                    cur = nxt
                nc.gpsimd.tensor_tensor(out=z.unsqueeze(-1), in0=cur[:, :, :, 0:1],
                                        in1=cur[:, :, :, 1:2],
                                        op=mybir.AluOpType.add)
            else:
                nc.vector.tensor_reduce(out=z, in_=e, axis=mybir.AxisListType.X,
                                        op=mybir.AluOpType.add)
            r = zpool.tile([128, SG, CH], F32, tag=f"r{g}")
            nc.vector.reciprocal(out=r, in_=z)
            if out_sep:
                cs = (cpoolA if g % 2 == 0 else cpoolB).tile(
                    [128, SG, CH, K], BF16, tag=f"cs{g}", name=f"cs{g}")
            else:
                cs = e
            mult_eng.tensor_tensor(out=cs, in0=e,
                                   in1=r.unsqueeze(-1).broadcast_to([128, SG, CH, K]),
                                   op=mybir.AluOpType.mult)
            return cs

        def s_matmuls(Y2):
            s_ps = ps_ys.tile([S, K, D], F32, tag="s")
            for k in range(K):
                nc.tensor.matmul(s_ps[:, k, :], Y2[:, :, k], wr[:, k, :],
                                 start=True, stop=True)
            return s_ps

        def squash_to_v(s_ps, quarter_done=None):
            vsb = vp.tile([S, K, D], F32, tag="v")
            _squash2(nc, sqp, s_ps, vsb, eps_t, quarter_done)
            return vsb

        def v_to_T2(vsb):
            # vT[d, k, s] via PE transposes; t-matmuls write [j, s, k] PSUM
            # directly (strided out) so the T2 cast is contiguous and the
            # b-update moving operand T2[:, s, :] is flat.
            vt_ps = ps_vt.tile([32, K, S], F32, tag="vt")
            for k in range(K):
                nc.tensor.transpose(out=vt_ps[:, k, :], in_=vsb[:, k, :],
                                    identity=ident)
            vt2 = vt2p.tile([32, K, S], BF16, tag="vt2")
            nc.scalar.copy(out=vt2, in_=vt_ps)
            t_ps = ps_t.tile([128, S, K], F32, tag="t")
            for k in range(K):
                nc.tensor.matmul(t_ps[:, :, k], wt[:, k, :],
                                 vt2[:, k, :], start=True, stop=True)
            T2 = t2p.tile([128, S, K], BF16, tag="T2")
            nc.vector.tensor_copy(out=T2, in_=t_ps)
            return T2

        def b_matmuls(s, T2):
            bu = ps_b.tile([128, CH, K], F32, tag="bu")
            for ic in range(CH):
                nc.tensor.matmul(bu[:, ic, :],
                                 xb_s[s][:, ic * 128:(ic + 1) * 128],
                                 T2[:, s, :], start=True, stop=True)
            return bu

        def y_matmuls(y_ps, cs, g):
            for si in range(SG):
                s = g * SG + si
                for ic in range(CH):
                    nc.tensor.matmul(y_ps[:, s, :], xa_s[s][:, ic, :],
                                     cs[:, si, ic, :],
                                     start=(ic == 0), stop=(ic == CH - 1))

        # ---------------- iteration 0 ----------------
        s_ps = ps_ys.tile([S, K, D], F32, tag="s")
        s_flat = s_ps.rearrange("s k d -> s (k d)")
        nc.tensor.matmul(s_flat[:, :512], y0t, wrf[:, :512], start=True, stop=True)
        nc.tensor.matmul(s_flat[:, 512:], y0t, wrf[:, 512:], start=True, stop=True)
        vsb = squash_to_v(s_ps)
        T2 = v_to_T2(vsb)
        for s in range(S):
            bu = b_matmuls(s, T2)
            dst = btile[:, s].rearrange("p c k -> p (c k)")
            buf = bu.rearrange("p c k -> p (c k)")
            # copies alternate DVE/ACT so the PE never waits on one engine
            if s % 2 == 0:
                nc.vector.tensor_copy(out=dst, in_=buf)
            else:
                nc.scalar.copy(out=dst, in_=buf)

        # ---------------- iteration 1: softmax + y, then s/squash/t ----------
        # normalize mult: GpSimd for the early groups (latency hides under
        # earlier work), DVE for the schedule-critical late groups.
        mult_engs = [nc.gpsimd, nc.gpsimd, nc.vector, nc.vector]
        y_ps = ps_ys.tile([128, S, K], F32, tag="y")
        e1 = []
        for g in range(NG):
            e = epool.tile([128, SG, CH, K], BF16, tag=f"e1{g}")
            cs = softmax_group(g, mult_engs[g], e_save=e)
            e1.append(e)
            y_matmuls(y_ps, cs, g)
        Y2 = y2p.tile([128, S, K], F32R, tag="Y2")
        nc.vector.tensor_copy(out=Y2, in_=y_ps)
        s_ps = s_matmuls(Y2)
        vsb = squash_to_v(s_ps)
        T2 = v_to_T2(vsb)

        # ------- iteration 1 b-update fused with iteration 2 softmax --------
        # exp(b0 + db) = exp(b0) * exp(db): ACT exps each sample's b-update
        # delta straight out of PSUM; one pure-bf16 2x DVE mult per group
        # recovers the unnormalized e for iteration 2. No adds, no copies.
        y_ps2 = ps_ys.tile([128, S, K], F32, tag="y")
        cs_q = []
        for g in range(NG):
            ed = edp.tile([128, SG, CH, K], BF16, tag=f"ed{g}")
            for si in range(SG):
                bu = b_matmuls(g * SG + si, T2)
                nc.scalar.activation(out=ed[:, si], in_=bu,
                                     func=mybir.ActivationFunctionType.Exp)
            e2 = (cpoolA if g % 2 == 0 else cpoolB).tile(
                [128, SG, CH, K], BF16, tag=f"cs{g}", name=f"cs{g}")
            nc.vector.tensor_tensor(out=e2, in0=e1[g], in1=ed,
                                    op=mybir.AluOpType.mult)
            cs_q.append(normalize(e2, g, mult_engs[g], out_sep=False))
            if g >= 1:
                y_matmuls(y_ps2, cs_q[g - 1], g - 1)
        y_matmuls(y_ps2, cs_q[NG - 1], NG - 1)

        # ---------------- iteration 2 tail: s, squash, output ----------------
        Y2b = y2p.tile([128, S, K], F32R, tag="Y2")
        nc.vector.tensor_copy(out=Y2b, in_=y_ps2)
        s_ps = s_matmuls(Y2b)
        outq = [nc.sync, nc.gpsimd]

        def stream_out(vt, k0, k1):
            outq[(k0 // (K // 4)) % 2].dma_start(out=v_d[:, k0:k1],
                                                 in_=vt[:, k0:k1])
        vsb = squash_to_v(s_ps, quarter_done=stream_out)

    nc.compile()
    return nc


def _get_program():
    global _PROGRAM
    if _PROGRAM is None:
        _PROGRAM = _build_program()
    return _PROGRAM


def _prep_core_inputs(x_core, wr, wt):
    """x_core: [S, I, J] fp32 -> per-core input map."""
    bf = ml_dtypes.bfloat16
    xa = np.ascontiguousarray(
        x_core.reshape(S, CH, 128, J).transpose(2, 0, 1, 3).astype(bf))  # [128,S,CH,J]
    xb = np.ascontiguousarray(x_core.transpose(2, 0, 1).astype(bf))      # [J,S,I]
    y0 = np.ascontiguousarray((x_core.sum(axis=1) / K).T)                # [J,S] f32
    return {"xa": xa, "xb": xb.reshape(J, S, CH * 128), "wr": wr,
            "wt": np.ascontiguousarray(wt.astype(bf)), "y0": y0}


def kernel(inputs, W):
    x = np.ascontiguousarray(np.asarray(inputs, dtype=np.float32))
    Wf = np.ascontiguousarray(np.asarray(W, dtype=np.float32))           # [J, K, D]
    wt = np.ascontiguousarray(Wf.transpose(2, 1, 0))                     # [D, K, J]
    nc = _get_program()
    in_maps = [_prep_core_inputs(x[c * S:(c + 1) * S], Wf, wt) for c in range(NCORES)]
    res = run_bass_kernel_spmd(nc, in_maps, list(range(NCORES)))
    return np.concatenate([r["vout"] for r in res.results], axis=0)


# revision 22
# speedup vs baseline: 1.0567x; 1.0231x over previous
"""Trainium2 Bass kernel for CapsuleLayer dynamic routing (B=128, I=1152, J=128, K=32, D=32).

Strategy
--------
Data-parallel over batch: 16 samples per core x 8 cores. The routing math is
algebraically factorized so u_hat [B,I,K,D] (604 MB) is never materialized:

    y[s,k,j]  = sum_i c[s,i,k] x[s,i,j]          (per-sample PE matmul, i contracted)
    s[s,k,d]  = sum_j y[s,k,j] W[j,k,d]          (per-k PE matmul, j contracted)
    v         = squash(s)                         (DVE/ACT elementwise)
    t[s,j,k]  = sum_d W[j,k,d] v[s,k,d]          (per-k PE matmul, d contracted)
    b[s,i,k] += sum_j x[s,i,j] t[s,j,k]          (per-sample PE matmul, j contracted)

x is staged in two on-chip layouts (i-major xa for y, j-major xb for the b
update), both prepared host-side in bf16. Perf-critical structure (from trace
analysis):
  - all big matmuls keep CONTIGUOUS moving operands (strided moving costs 60ns
    vs 27ns per instr): cs is k-inner; the t-matmul writes its PSUM output
    pre-transposed ([j, s, k]) so T2[:, s, :] is flat for the b-update.
  - t-matmuls run in bf16 (f32r stationary ldweights cost 328ns each).
  - iteration 1's agreement update never touches b again: iteration 2 uses
    exp(b0 + db) = exp(b0) * exp(db), with exp(db) read straight out of the
    b-update PSUM tile by the ACT engine per sample — no PSUM->SBUF copies or
    adds on the DVE.
  - squash's two k-halves are op-interleaved so ACT/DVE pipeline, and the
    Sqrt table load hides under the DVE reduce.
  - softmax normalize mult runs on DVE for the schedule-critical groups
    (g0, g3) and GpSimd for the middle ones.
  - x DMAs ride 3 hardware queues (sync/gpsimd/scalar); all xb tiles land
    first (iteration 0's b-update + iteration 1's softmax complete early),
    then xa group by group (iteration 1's y rides the tail).
Iteration 0's softmax of zeros is uniform, so its y reduces to sum_i x / K,
computed host-side in fp32 (y0).
"""
import numpy as np
import ml_dtypes
from contextlib import ExitStack

import concourse.bass as bass
import concourse.bacc as bacc_mod
import concourse.mybir as mybir
import concourse.tile as tile
from concourse.bass_utils import run_bass_kernel_spmd
from concourse.masks import make_identity

B, I, J, K, D = 128, 1152, 128, 32, 32
NCORES = 8
S = B // NCORES          # 16 samples per core
CH = I // 128            # 9 chunks of the input-capsule axis
NUM_ROUTING = 3
EPS = 1e-7
F32 = mybir.dt.float32
F32R = mybir.dt.float32r
BF16 = mybir.dt.bfloat16

_PROGRAM = None

SG = 4  # softmax sample-group size
NG = S // SG


def _squash2(nc, sqpool, s_ps, vsb, eps_t, quarter_done=None):
    """vsb = squash(s_ps) along d. The PSUM Square runs in two k-halves on
    ACT; everything downstream runs in four k-quarters so the ACT/DVE chains
    pipeline (and the Sqrt table load hides under the reduces).
    quarter_done(k0, k1) is called after each quarter's vsb slice is final."""
    sq = {}
    for k0 in (0, K // 2):
        k1 = k0 + K // 2
        sq[k0] = sqpool.tile([S, K // 2, D], F32, tag=f"sq{k0}", name=f"sq{k0}")
        nc.scalar.activation(out=sq[k0], in_=s_ps[:, k0:k1, :],
                             func=mybir.ActivationFunctionType.Square)
    KQ = K // 4
    quarters = [(q * KQ, (q + 1) * KQ) for q in range(4)]
    ss, rt = {}, {}
    for k0, k1 in quarters:
        sqh = sq[0 if k0 < K // 2 else K // 2]
        o = k0 % (K // 2)
        ss[k0] = sqpool.tile([S, KQ], F32, tag=f"ss{k0}", name=f"ss{k0}")
        nc.vector.tensor_reduce(out=ss[k0], in_=sqh[:, o:o + KQ],
                                axis=mybir.AxisListType.X, op=mybir.AluOpType.add)
    for k0, k1 in quarters:
        rt[k0] = sqpool.tile([S, KQ], F32, tag=f"rt{k0}", name=f"rt{k0}")
        nc.scalar.activation(out=rt[k0], in_=ss[k0],
                             func=mybir.ActivationFunctionType.Sqrt, bias=eps_t)
    for k0, k1 in quarters:
        den = sqpool.tile([S, KQ], F32, tag=f"den{k0}", name=f"den{k0}")
        nc.vector.scalar_tensor_tensor(out=den, in0=ss[k0], scalar=1.0,
                                       in1=rt[k0], op0=mybir.AluOpType.add,
                                       op1=mybir.AluOpType.mult)
        rden = sqpool.tile([S, KQ], F32, tag=f"rden{k0}", name=f"rden{k0}")
        nc.vector.reciprocal(out=rden, in_=den)
        sc = sqpool.tile([S, KQ], F32, tag=f"sc{k0}", name=f"sc{k0}")
        nc.vector.tensor_mul(sc, ss[k0], rden)
        nc.vector.tensor_tensor(out=vsb[:, k0:k1, :], in0=s_ps[:, k0:k1, :],
                                in1=sc.unsqueeze(-1).broadcast_to([S, KQ, D]),
                                op=mybir.AluOpType.mult)
        if quarter_done is not None:
            quarter_done(vsb, k0, k1)


def _build_program():
    nc = bacc_mod.Bacc("TRN2", target_bir_lowering=False, debug=False,
                       num_devices=NCORES)
    xa_d = nc.dram_tensor("xa", [128, S, CH, 128], BF16, kind="ExternalInput")
    xb_d = nc.dram_tensor("xb", [128, S, CH * 128], BF16, kind="ExternalInput")
    wr_d = nc.dram_tensor("wr", [128, K, D], F32R, kind="ExternalInput")
    wt_d = nc.dram_tensor("wt", [32, K, 128], BF16, kind="ExternalInput")
    y0_d = nc.dram_tensor("y0", [128, S], F32R, kind="ExternalInput")
    v_d = nc.dram_tensor("vout", [S, K, D], F32, kind="ExternalOutput")

    with tile.TileContext(nc) as tc, ExitStack() as ctx:
        # pool order = SBUF address order: tiles feeding DVE 2x bf16 ops
        # (e1, ed, cs) must sit below the 128KB per-partition boundary —
        # above it the paired-fetch mode degrades ~2.4x (measured).
        const = ctx.enter_context(tc.tile_pool(name="const", bufs=1))
        epool = ctx.enter_context(tc.tile_pool(name="e1", bufs=1))
        edp = ctx.enter_context(tc.tile_pool(name="ed", bufs=1))
        cpool = ctx.enter_context(tc.tile_pool(name="c", bufs=1))
        zpool = ctx.enter_context(tc.tile_pool(name="z", bufs=1))
        y2p = ctx.enter_context(tc.tile_pool(name="y2", bufs=2))
        t2p = ctx.enter_context(tc.tile_pool(name="t2", bufs=2))
        vt2p = ctx.enter_context(tc.tile_pool(name="vt2", bufs=2))
        vp = ctx.enter_context(tc.tile_pool(name="v", bufs=1))
        sqp = ctx.enter_context(tc.tile_pool(name="sqp", bufs=1))
        xap = ctx.enter_context(tc.tile_pool(name="xa", bufs=1))
        xbp = ctx.enter_context(tc.tile_pool(name="xb", bufs=1))
        bp = ctx.enter_context(tc.tile_pool(name="b", bufs=1))
        ps_ys = ctx.enter_context(tc.tile_pool(name="ps_ys", bufs=1, space="PSUM"))
        ps_vt = ctx.enter_context(tc.tile_pool(name="ps_vt", bufs=1, space="PSUM"))
        ps_t = ctx.enter_context(tc.tile_pool(name="ps_t", bufs=1, space="PSUM"))
        ps_b = ctx.enter_context(tc.tile_pool(name="ps_b", bufs=3, space="PSUM"))

        # constants: small, needed first for iteration 0's s/t chain.
        y0t = const.tile([128, S], F32R)
        nc.sync.dma_start(out=y0t, in_=y0_d[:])
        wr = const.tile([128, K, D], F32R)
        nc.gpsimd.dma_start(out=wr, in_=wr_d[:])
        wt = const.tile([32, K, 128], BF16)
        nc.scalar.dma_start(out=wt, in_=wt_d[:])
        ident = const.tile([S, S], F32)
        make_identity(nc, ident)
        eps_t = const.tile([S, 1], F32)
        nc.vector.memset(eps_t, EPS)

        # x loads: ALL xb first (b-update it0 finishes early, freeing DVE/ACT
        # before iteration 1's softmax), then xa group by group. Only the
        # sync/gpsimd queues carry x tiles — a full DGE ring stalls the
        # issuing engine, and the scalar (ACT) engine has early squash work.
        dq = [nc.sync, nc.gpsimd]
        xb_s = [None] * S
        xa_s = [None] * S
        for s in range(S):
            tb = xbp.tile([128, CH * 128], BF16, tag=f"xb{s}")
            dq[s % 2].dma_start(out=tb, in_=xb_d[:, s])
            xb_s[s] = tb
        for s in range(S):
            ta = xap.tile([128, CH, 128], BF16, tag=f"xa{s}")
            dq[s % 2].dma_start(out=ta, in_=xa_d[:, s])
            xa_s[s] = ta

        btile = bp.tile([128, S, CH, K], F32)
        wrf = wr.rearrange("p k d -> p (k d)")

        def softmax_group(g, mult_eng, e_save=None):
            """softmax over k of b[:, gSG:(g+1)SG] -> bf16 cs tile.
            If e_save is given, exp() lands there (kept for iteration 2) and
            the normalized cs goes to a separate tile."""
            bsl = btile[:, g * SG:(g + 1) * SG]
            if e_save is not None:
                e = e_save
            else:
                e = cpool.tile([128, SG, CH, K], BF16, tag=f"cs{g}", name=f"cs{g}")
            nc.scalar.activation(out=e, in_=bsl,
                                 func=mybir.ActivationFunctionType.Exp)
            return normalize(e, g, mult_eng, out_sep=e_save is not None)

        def normalize(e, g, mult_eng, out_sep, red_gps=False):
            z = zpool.tile([128, SG, CH], F32, tag=f"z{g}")
            if red_gps:
                # halving-tree add on GpSimd (bf16 intermediates) frees the
                # DVE of the 1.35us reduce in the fused region
                cur, w = e, K
                while w > 2:
                    w //= 2
                    nxt = zpool.tile([128, SG, CH, w], BF16, tag=f"zt{g}_{w}",
                                     name=f"zt{w}")
                    nc.gpsimd.tensor_tensor(out=nxt, in0=cur[:, :, :, 0:w],
                                            in1=cur[:, :, :, w:2 * w],
                                            op=mybir.AluOpType.add)
                    cur = nxt
                nc.gpsimd.tensor_tensor(out=z.unsqueeze(-1), in0=cur[:, :, :, 0:1],
                                        in1=cur[:, :, :, 1:2],
                                        op=mybir.AluOpType.add)
            else:
                nc.vector.tensor_reduce(out=z, in_=e, axis=mybir.AxisListType.X,
                                        op=mybir.AluOpType.add)
            r = zpool.tile([128, SG, CH], F32, tag=f"r{g}")
            nc.vector.reciprocal(out=r, in_=z)
            if out_sep:
                cs = cpool.tile([128, SG, CH, K], BF16, tag=f"cs{g}", name=f"cs{g}")
            else:
                cs = e
            mult_eng.tensor_tensor(out=cs, in0=e,
                                   in1=r.unsqueeze(-1).broadcast_to([128, SG, CH, K]),
                                   op=mybir.AluOpType.mult)
            return cs

        def s_matmuls(Y2):
            s_ps = ps_ys.tile([S, K, D], F32, tag="s")
            for k in range(K):
                nc.tensor.matmul(s_ps[:, k, :], Y2[:, :, k], wr[:, k, :],
                                 start=True, stop=True)
            return s_ps

        def squash_to_v(s_ps, quarter_done=None):
            vsb = vp.tile([S, K, D], F32, tag="v")
            _squash2(nc, sqp, s_ps, vsb, eps_t, quarter_done)
            return vsb

        def v_to_T2(vsb):
            # vT[d, k, s] via PE transposes; t-matmuls write [j, s, k] PSUM
            # directly (strided out) so the T2 cast is contiguous and the
            # b-update moving operand T2[:, s, :] is flat.
            vt_ps = ps_vt.tile([32, K, S], F32, tag="vt")
            for k in range(K):
                nc.tensor.transpose(out=vt_ps[:, k, :], in_=vsb[:, k, :],
                                    identity=ident)
            vt2 = vt2p.tile([32, K, S], BF16, tag="vt2")
            nc.scalar.copy(out=vt2, in_=vt_ps)
            t_ps = ps_t.tile([128, S, K], F32, tag="t")
            for k in range(K):
                nc.tensor.matmul(t_ps[:, :, k], wt[:, k, :],
                                 vt2[:, k, :], start=True, stop=True)
            T2 = t2p.tile([128, S, K], BF16, tag="T2")
            nc.vector.tensor_copy(out=T2, in_=t_ps)
            return T2

        def b_matmuls(s, T2):
            bu = ps_b.tile([128, CH, K], F32, tag="bu")
            for ic in range(CH):
                nc.tensor.matmul(bu[:, ic, :],
                                 xb_s[s][:, ic * 128:(ic + 1) * 128],
                                 T2[:, s, :], start=True, stop=True)
            return bu

        def y_matmuls(y_ps, cs, g):
            for si in range(SG):
                s = g * SG + si
                for ic in range(CH):
                    nc.tensor.matmul(y_ps[:, s, :], xa_s[s][:, ic, :],
                                     cs[:, si, ic, :],
                                     start=(ic == 0), stop=(ic == CH - 1))

        # ---------------- iteration 0 ----------------
        s_ps = ps_ys.tile([S, K, D], F32, tag="s")
        s_flat = s_ps.rearrange("s k d -> s (k d)")
        nc.tensor.matmul(s_flat[:, :512], y0t, wrf[:, :512], start=True, stop=True)
        nc.tensor.matmul(s_flat[:, 512:], y0t, wrf[:, 512:], start=True, stop=True)
        vsb = squash_to_v(s_ps)
        T2 = v_to_T2(vsb)
        for s in range(S):
            bu = b_matmuls(s, T2)
            dst = btile[:, s].rearrange("p c k -> p (c k)")
            buf = bu.rearrange("p c k -> p (c k)")
            # copies alternate DVE/ACT so the PE never waits on one engine
            if s % 2 == 0:
                nc.vector.tensor_copy(out=dst, in_=buf)
            else:
                nc.scalar.copy(out=dst, in_=buf)

        # ---------------- iteration 1: softmax + y, then s/squash/t ----------
        # normalize mult: GpSimd for the early groups (latency hides under
        # earlier work), DVE for the schedule-critical late groups.
        mult_engs = [nc.gpsimd, nc.gpsimd, nc.vector, nc.vector]
        y_ps = ps_ys.tile([128, S, K], F32, tag="y")
        e1 = []
        for g in range(NG):
            e = epool.tile([128, SG, CH, K], BF16, tag=f"e1{g}")
            cs = softmax_group(g, mult_engs[g], e_save=e)
            e1.append(e)
            y_matmuls(y_ps, cs, g)
        Y2 = y2p.tile([128, S, K], F32R, tag="Y2")
        nc.vector.tensor_copy(out=Y2, in_=y_ps)
        s_ps = s_matmuls(Y2)
        vsb = squash_to_v(s_ps)
        T2 = v_to_T2(vsb)

        # ------- iteration 1 b-update fused with iteration 2 softmax --------
        # exp(b0 + db) = exp(b0) * exp(db): ACT exps each sample's b-update
        # delta straight out of PSUM; one pure-bf16 2x DVE mult per group
        # recovers the unnormalized e for iteration 2. No adds, no copies.
        y_ps2 = ps_ys.tile([128, S, K], F32, tag="y")
        cs_q = []
        for g in range(NG):
            ed = edp.tile([128, SG, CH, K], BF16, tag=f"ed{g}")
            for si in range(SG):
                bu = b_matmuls(g * SG + si, T2)
                nc.scalar.activation(out=ed[:, si], in_=bu,
                                     func=mybir.ActivationFunctionType.Exp)
            e2 = cpool.tile([128, SG, CH, K], BF16, tag=f"cs{g}", name=f"cs{g}")
            nc.vector.tensor_tensor(out=e2, in0=e1[g], in1=ed,
                                    op=mybir.AluOpType.mult)
            cs_q.append(normalize(e2, g, mult_engs[g], out_sep=False))
            if g >= 1:
                y_matmuls(y_ps2, cs_q[g - 1], g - 1)
        y_matmuls(y_ps2, cs_q[NG - 1], NG - 1)

        # ---------------- iteration 2 tail: s, squash, output ----------------
        Y2b = y2p.tile([128, S, K], F32R, tag="Y2")
        nc.vector.tensor_copy(out=Y2b, in_=y_ps2)
        s_ps = s_matmuls(Y2b)
        outq = [nc.sync, nc.gpsimd]

        def stream_out(vt, k0, k1):
            outq[(k0 // (K // 4)) % 2].dma_start(out=v_d[:, k0:k1],
                                                 in_=vt[:, k0:k1])
        vsb = squash_to_v(s_ps, quarter_done=stream_out)

    nc.compile()
    return nc


def _get_program():
    global _PROGRAM
    if _PROGRAM is None:
        _PROGRAM = _build_program()
    return _PROGRAM


def _prep_core_inputs(x_core, wr, wt):
    """x_core: [S, I, J] fp32 -> per-core input map."""
    bf = ml_dtypes.bfloat16
    xa = np.ascontiguousarray(
        x_core.reshape(S, CH, 128, J).transpose(2, 0, 1, 3).astype(bf))  # [128,S,CH,J]
    xb = np.ascontiguousarray(x_core.transpose(2, 0, 1).astype(bf))      # [J,S,I]
    y0 = np.ascontiguousarray((x_core.sum(axis=1) / K).T)                # [J,S] f32
    return {"xa": xa, "xb": xb.reshape(J, S, CH * 128), "wr": wr,
            "wt": np.ascontiguousarray(wt.astype(bf)), "y0": y0}


def kernel(inputs, W):
    x = np.ascontiguousarray(np.asarray(inputs, dtype=np.float32))
    Wf = np.ascontiguousarray(np.asarray(W, dtype=np.float32))           # [J, K, D]
    wt = np.ascontiguousarray(Wf.transpose(2, 1, 0))                     # [D, K, J]
    nc = _get_program()
    in_maps = [_prep_core_inputs(x[c * S:(c + 1) * S], Wf, wt) for c in range(NCORES)]
    res = run_bass_kernel_spmd(nc, in_maps, list(range(NCORES)))
    return np.concatenate([r["vout"] for r in res.results], axis=0)
